# revision 1
# baseline (speedup 1.0000x reference)
"""Trainium2 Bass kernel for the 2-graph GCN (nn_Net_39041252721058).

Strategy (8 NeuronCores, SPMD single program):
  - Core k owns dst nodes [k*6250, (k+1)*6250). All edges with dst in that
    range are processed by core k, grouped by 128-node dst blocks.
  - Layer math uses the linearity of spmm: project first (x@W1 row-sharded,
    AllGather of the projected bf16 table), then per-edge gather rows of the
    table with SWDGE dma_gather (int16 indices; src split in two classes at
    row 32768 so indices fit), then segment-sum via one-hot matmuls on the
    tensor engine accumulating in PSUM (edges on the contraction axis,
    one-hot dst-slot selection matrices built on the vector engine).
  - h = relu(agg + b1) stays on-chip per block; support2 = h @ W2 uses PE
    transposes of h; AllGather of support2, then the same gather/scatter
    machinery for layer 2.
  - The two graphs' phases are interleaved (A_d, AG1_d, A_s, AG1_s, B_d,
    AG2_d, B_s, AG2_s, C_d, C_s) so all collectives except the first hide
    behind the other graph's gather stream.
  - Phase A interleaves two PSUM accumulators (even/odd k-tiles) so PE
    weight loads pipeline; the halves are combined via a scalar-engine
    PSUM->SBUF copy plus a vector add (a vector op may read only one PSUM
    operand).

Perf model (measured on trn2): the kernel is bound by the SWDGE gather
ucode/ring, ~135 ns per 16-row ring entry plus ~0.7 us per dma_gather
call, insensitive to row bytes and to index order. Optimizations applied,
in order of measured impact:
  - gather calls batched to 1536 rows (96 of the ring's 128 entries;
    >=184 entries crashes the ucode, so one call per class is impossible),
  - per-BLOCK gather counts (ceil-16 of the max count over the 8 cores,
    computed at compile time; the SPMD constraint is across cores, not
    across the unrolled block loop),
  - graph-interleaved phases, dual-PSUM phase A, single_packet=False
    (True crashes the ucode at 1536-row calls; it only works at 512).
Rows gathered are ~3.5%% above the true edge count; driving num_idxs_reg
from per-core registers to close that gap crashes the runtime (the
completion-semaphore contract appears to assume the static count).
Everything else (PE one-hot matmuls, vector sel builds, collectives 2-4)
overlaps fully under the gather stream; the only exposed non-gather time
is phase A_d plus the first AllGather (~150 us).

Correctness details: pads carry w=0 so the one-hot multiply zeroes them;
message buffers are memset once on first use so never-gathered pad slots
cannot inject NaN (0*NaN = NaN would poison PSUM).
"""
import numpy as np
import ml_dtypes

import concourse.bass as bass
import concourse.bacc as bacc
import concourse.mybir as mybir
import concourse.tile as tile
from concourse.bass_utils import run_bass_kernel_spmd

NCORES = 8
NODES = 50000
PER_CORE = NODES // NCORES           # 6250
NBLK = (PER_CORE + 127) // 128       # 49 (last block has 106 nodes)
SPLIT = 32768                        # int16 gather-index class boundary
F_IN = 512
H1 = 256
H2 = 128

BF16 = ml_dtypes.bfloat16


def _set_dims(nodes, split):
    """Debug hook: shrink the problem (node count) keeping the structure."""
    global NODES, PER_CORE, NBLK, SPLIT
    NODES = nodes
    PER_CORE = NODES // NCORES
    NBLK = (PER_CORE + 127) // 128
    SPLIT = split


# ----------------------------------------------------------------------------
# Host-side edge preprocessing
# ----------------------------------------------------------------------------

def _analyze(src, dst):
    core = dst // PER_CORE
    blk = (dst % PER_CORE) // 128
    cls = (src >= SPLIT).astype(np.int64)
    key = (core * NBLK + blk) * 2 + cls
    counts = np.bincount(key, minlength=NCORES * NBLK * 2).reshape(-1, 2)
    return int(counts[:, 0].max()), int(counts[:, 1].max())


def _analyze_blocks(src, dst):
    """Per-block max-over-cores counts, ceil-16, per class: [NBLK, 2] int."""
    core = dst // PER_CORE
    blk = (dst % PER_CORE) // 128
    cls = (src >= SPLIT).astype(np.int64)
    key = (core * NBLK + blk) * 2 + cls
    counts = np.bincount(key, minlength=NCORES * NBLK * 2)
    counts = counts.reshape(NCORES, NBLK, 2).max(axis=0)
    return np.maximum(16, -(-counts // 16) * 16)


def _prep_graph(src, dst, w, G_A, G_B):
    """Per-core gather index / one-hot position / weight arrays (padded)."""
    GT = G_A + G_B
    core_all = dst // PER_CORE
    out = []
    for k in range(NCORES):
        m = core_all == k
        s, ww = src[m], w[m]
        rel = dst[m] - k * PER_CORE
        blk = rel // 128
        slot = rel % 128
        cls = (s >= SPLIT).astype(np.int64)
        order = np.lexsort((s, cls, blk))
        s, ww, blk, slot, cls = s[order], ww[order], blk[order], slot[order], cls[order]

        idxA = np.zeros((NBLK, G_A * 128), np.int16)
        idxB = np.zeros((NBLK, G_B * 128), np.int16)
        pos = np.zeros((NBLK, GT * 128), np.float32)
        wv = np.zeros((NBLK, GT * 128), np.float32)
        key = blk * 2 + cls
        cnt = np.bincount(key, minlength=NBLK * 2).reshape(NBLK, 2)
        assert cnt[:, 0].max() <= G_A * 128 and cnt[:, 1].max() <= G_B * 128
        starts = np.concatenate([[0], np.cumsum(cnt.ravel())])
        for b in range(NBLK):
            nA, nB = cnt[b, 0], cnt[b, 1]
            oA, oB = starts[b * 2], starts[b * 2 + 1]
            idxA[b, :nA] = s[oA:oA + nA]
            idxB[b, :nB] = s[oB:oB + nB] - SPLIT
            pos[b, :nA] = slot[oA:oA + nA]
            pos[b, G_A * 128:G_A * 128 + nB] = slot[oB:oB + nB]
            wv[b, :nA] = ww[oA:oA + nA]
            wv[b, G_A * 128:G_A * 128 + nB] = ww[oB:oB + nB]

        def wrap_idx(a):
            nb, n = a.shape
            b16 = a.reshape(nb, n // 16, 16).transpose(2, 0, 1).reshape(16, -1)
            return np.tile(b16, (8, 1)).copy()

        def wrap_edge(a):
            nb, n = a.shape
            return (a.reshape(nb, n // 128, 128).transpose(2, 0, 1)
                    .reshape(128, -1).astype(BF16).copy())

        out.append({
            "idxA": wrap_idx(idxA),
            "idxB": wrap_idx(idxB),
            "pos": wrap_edge(pos),
            "wv": wrap_edge(wv),
        })
    return out


def _prep_x(x, k):
    """Blocked transposed node features for core k: [NBLK*128, F_IN] bf16
    with row b*128+i, col kt*128+j = x[k*PER_CORE + b*128 + j, kt*128 + i]."""
    xs = np.zeros((NBLK * 128, F_IN), BF16)
    xk = x[k * PER_CORE:(k + 1) * PER_CORE].astype(BF16)  # [6250, 512]
    for b in range(NBLK):
        rows = min(128, PER_CORE - b * 128)
        blkT = xk[b * 128:b * 128 + rows].T  # [512, rows]
        t = blkT.reshape(4, 128, rows)       # [kt, i, j]
        xs[b * 128:(b + 1) * 128, :] = np.transpose(
            np.pad(t, ((0, 0), (0, 0), (0, 128 - rows))), (1, 0, 2)
        ).reshape(128, 512)
    return xs


# ----------------------------------------------------------------------------
# Device program
# ----------------------------------------------------------------------------

def _chunks16(total):
    """Split `total` (multiple of 16) gather rows into calls of <=1536 rows
    (96 ring entries of 16 rows; the SWDGE ring holds 128)."""
    out, r0 = [], 0
    while r0 < total:
        n = min(1536, total - r0)
        out.append((r0, n))
        r0 += n
    return out


def _graph_setup(nc, tc, sb, ps, dr, p, G_A, G_B, tens, consts,
                 N_A=None, N_B=None):
    """Load resident tiles + alloc DRAM intermediates for one graph.
    N_A/N_B: per-block gather row counts [NBLK] (ceil-16 max over cores)."""
    if N_A is None:
        N_A = [G_A * 128] * NBLK
        N_B = [G_B * 128] * NBLK
    GT = G_A + G_B
    dt = mybir.dt
    iota_t, ident_t, ones_t = consts

    # resident per-graph tiles
    w1_t = sb.tile([128, 4, H1], dt.bfloat16, tag="w1")
    nc.sync.dma_start(out=w1_t[:], in_=tens[p + "W1"][:].rearrange("(a b) c -> b a c", b=128))
    w2_t = sb.tile([128, 2, H2], dt.bfloat16, tag="w2")
    nc.sync.dma_start(out=w2_t[:], in_=tens[p + "W2"][:].rearrange("(a b) c -> b a c", b=128))
    b1_t = sb.tile([1, H1], dt.bfloat16, tag="b1")
    nc.sync.dma_start(out=b1_t[:], in_=tens[p + "b1"][:])
    b2_t = sb.tile([1, H2], dt.bfloat16, tag="b2")
    nc.sync.dma_start(out=b2_t[:], in_=tens[p + "b2"][:])
    idxA_t = sb.tile([128, NBLK * G_A * 8], dt.int16, tag="idxA")
    nc.sync.dma_start(out=idxA_t[:], in_=tens[p + "idxA"][:])
    idxB_t = sb.tile([128, NBLK * G_B * 8], dt.int16, tag="idxB")
    nc.sync.dma_start(out=idxB_t[:], in_=tens[p + "idxB"][:])
    wv_t = sb.tile([128, NBLK * GT], dt.bfloat16, tag="wv")
    nc.sync.dma_start(out=wv_t[:], in_=tens[p + "wv"][:])
    pos_t = sb.tile([128, NBLK * GT], dt.bfloat16, tag="pos")
    nc.sync.dma_start(out=pos_t[:], in_=tens[p + "pos"][:])

    # DRAM intermediates
    s1_own = dr.tile([PER_CORE, H1], dt.bfloat16, tag=p + "s1o")
    s1_full = dr.tile([NODES, H1], dt.bfloat16, tag=p + "s1f", addr_space="Shared")
    s2_own = dr.tile([PER_CORE, H2], dt.bfloat16, tag=p + "s2o")
    s2_full = dr.tile([NODES, H2], dt.bfloat16, tag=p + "s2f", addr_space="Shared")

    return dict(locals())


def _phase_A(st):
    nc, sb, ps, p, tens = st["nc"], st["sb"], st["ps"], st["p"], st["tens"]
    dt = mybir.dt
    w1_t, s1_own = st["w1_t"], st["s1_own"]
    # ---- Phase A: support1 = x @ W1 (own rows) ----
    for b in range(NBLK):
        rows = min(128, PER_CORE - b * 128)
        xt = sb.tile([128, 4, 128], dt.bfloat16, tag="xt")
        nc.sync.dma_start(
            out=xt[:],
            in_=tens[p + "xT"][b * 128:(b + 1) * 128, :].rearrange("p (a c) -> p a c", a=4),
        )
        accE = ps.tile([128, H1], dt.float32, tag="acc256")
        accO = ps.tile([128, H1], dt.float32, tag="acc256b")
        for kt in range(4):
            a = accE if kt % 2 == 0 else accO
            nc.tensor.matmul(a[:], lhsT=xt[:, kt, :], rhs=w1_t[:, kt, :],
                             start=(kt < 2), stop=(kt >= 2))
        tA = sb.tile([128, H1], dt.bfloat16, tag="tA", name="tA")
        nc.scalar.activation(tA[:], accO[:], mybir.ActivationFunctionType.Copy)
        s1sb = sb.tile([128, H1], dt.bfloat16, tag="s1sb")
        nc.vector.tensor_tensor(out=s1sb[:], in0=accE[:], in1=tA[:],
                                op=mybir.AluOpType.add)
        nc.sync.dma_start(out=s1_own[b * 128:b * 128 + rows, :], in_=s1sb[:rows, :])

    nc.gpsimd.collective_compute(
        "AllGather", mybir.AluOpType.bypass,
        replica_groups=[list(range(NCORES))],
        ins=[s1_own.opt()], outs=[st["s1_full"].opt()],
    )


def _phase_B(st):
    nc, sb, ps, p, tens = st["nc"], st["sb"], st["ps"], st["p"], st["tens"]
    dt = mybir.dt
    G_A, G_B, GT = st["G_A"], st["G_B"], st["GT"]
    N_A, N_B = st["N_A"], st["N_B"]
    iota_t, ident_t, ones_t = st["iota_t"], st["ident_t"], st["ones_t"]
    idxA_t, idxB_t, pos_t, wv_t = st["idxA_t"], st["idxB_t"], st["pos_t"], st["wv_t"]
    w2_t, b1_t = st["w2_t"], st["b1_t"]
    s1_full, s2_own = st["s1_full"], st["s2_own"]
    # ---- Phase B: agg1 -> h -> support2 (own rows) ----
    for b in range(NBLK):
        rows = min(128, PER_CORE - b * 128)
        msgsA = sb.tile([128, G_A, H1], dt.bfloat16, tag="mA")
        if b < 2:
            nc.vector.memset(msgsA[:], 0.0)
        for r0, n in _chunks16(int(N_A[b])):
            g0, g1 = r0 // 128, (r0 + n + 127) // 128
            nc.gpsimd.dma_gather(
                msgsA[:, g0:g1, :], s1_full[:],
                idxA_t[:, b * G_A * 8 + r0 // 16:
                       b * G_A * 8 + r0 // 16 + -(-n // 16)],
                n, n, H1, single_packet=False)
        msgsB = sb.tile([128, G_B, H1], dt.bfloat16, tag="mB")
        if b < 2:
            nc.vector.memset(msgsB[:], 0.0)
        for r0, n in _chunks16(int(N_B[b])):
            g0, g1 = r0 // 128, (r0 + n + 127) // 128
            nc.gpsimd.dma_gather(
                msgsB[:, g0:g1, :], s1_full[SPLIT:, :],
                idxB_t[:, b * G_B * 8 + r0 // 16:
                       b * G_B * 8 + r0 // 16 + -(-n // 16)],
                n, n, H1, single_packet=False)

        posb = pos_t[:, b * GT:(b + 1) * GT]
        wvb = wv_t[:, b * GT:(b + 1) * GT]
        ia = iota_t[:, :]
        iota_b = bass.AP(tensor=ia.tensor, offset=ia.offset,
                         ap=[ia.ap[0], [0, GT], ia.ap[1]])
        eq = sb.tile([128, GT, 128], dt.bfloat16, tag="eq")
        nc.vector.tensor_tensor(out=eq[:], in0=iota_b,
                                in1=posb.to_broadcast([128, GT, 128]),
                                op=mybir.AluOpType.is_equal)
        sel = sb.tile([128, GT, 128], dt.bfloat16, tag="sel")
        nc.vector.tensor_tensor(out=sel[:], in0=eq[:],
                                in1=wvb.to_broadcast([128, GT, 128]),
                                op=mybir.AluOpType.mult)

        acc = ps.tile([128, H1], dt.float32, tag="acc256")
        nc.tensor.matmul(acc[:], lhsT=ones_t[:], rhs=b1_t[:], start=True, stop=False)
        for c in range(G_A):
            nc.tensor.matmul(acc[:], lhsT=sel[:, c, :], rhs=msgsA[:, c, :],
                             start=False, stop=False)
        for c in range(G_B):
            nc.tensor.matmul(acc[:], lhsT=sel[:, G_A + c, :], rhs=msgsB[:, c, :],
                             start=False, stop=(c == G_B - 1))

        h_bf = sb.tile([128, H1], dt.bfloat16, tag="hbf")
        nc.scalar.activation(h_bf[:], acc[:], mybir.ActivationFunctionType.Relu)

        sp2 = ps.tile([128, H2], dt.float32, tag="acc128")
        for half in range(2):
            tp = ps.tile([128, 128], dt.bfloat16, tag="tp")
            nc.tensor.transpose(out=tp[:], in_=h_bf[:, half * 128:(half + 1) * 128],
                                identity=ident_t[:])
            hT = sb.tile([128, 128], dt.bfloat16, tag="hT")
            nc.vector.tensor_copy(out=hT[:], in_=tp[:])
            nc.tensor.matmul(sp2[:], lhsT=hT[:], rhs=w2_t[:, half, :],
                             start=(half == 0), stop=(half == 1))
        s2sb = sb.tile([128, H2], dt.bfloat16, tag="s2sb")
        nc.vector.tensor_copy(out=s2sb[:], in_=sp2[:])
        nc.sync.dma_start(out=s2_own[b * 128:b * 128 + rows, :], in_=s2sb[:rows, :])

    nc.gpsimd.collective_compute(
        "AllGather", mybir.AluOpType.bypass,
        replica_groups=[list(range(NCORES))],
        ins=[s2_own.opt()], outs=[st["s2_full"].opt()],
    )


def _phase_C(st):
    nc, sb, ps, p, tens = st["nc"], st["sb"], st["ps"], st["p"], st["tens"]
    dt = mybir.dt
    G_A, G_B, GT = st["G_A"], st["G_B"], st["GT"]
    N_A, N_B = st["N_A"], st["N_B"]
    iota_t, ones_t = st["iota_t"], st["ones_t"]
    idxA_t, idxB_t, pos_t, wv_t = st["idxA_t"], st["idxB_t"], st["pos_t"], st["wv_t"]
    b2_t = st["b2_t"]
    s2_full = st["s2_full"]
    # ---- Phase C: agg2 + b2 -> out ----
    for b in range(NBLK):
        rows = min(128, PER_CORE - b * 128)
        msgsA = sb.tile([128, G_A, H2], dt.bfloat16, tag="mA")
        if b < 2:
            nc.vector.memset(msgsA[:], 0.0)
        for r0, n in _chunks16(int(N_A[b])):
            g0, g1 = r0 // 128, (r0 + n + 127) // 128
            nc.gpsimd.dma_gather(
                msgsA[:, g0:g1, :], s2_full[:],
                idxA_t[:, b * G_A * 8 + r0 // 16:
                       b * G_A * 8 + r0 // 16 + -(-n // 16)],
                n, n, H2, single_packet=False)
        msgsB = sb.tile([128, G_B, H2], dt.bfloat16, tag="mB")
        if b < 2:
            nc.vector.memset(msgsB[:], 0.0)
        for r0, n in _chunks16(int(N_B[b])):
            g0, g1 = r0 // 128, (r0 + n + 127) // 128
            nc.gpsimd.dma_gather(
                msgsB[:, g0:g1, :], s2_full[SPLIT:, :],
                idxB_t[:, b * G_B * 8 + r0 // 16:
                       b * G_B * 8 + r0 // 16 + -(-n // 16)],
                n, n, H2, single_packet=False)

        posb = pos_t[:, b * GT:(b + 1) * GT]
        wvb = wv_t[:, b * GT:(b + 1) * GT]
        ia = iota_t[:, :]
        iota_b = bass.AP(tensor=ia.tensor, offset=ia.offset,
                         ap=[ia.ap[0], [0, GT], ia.ap[1]])
        eq = sb.tile([128, GT, 128], dt.bfloat16, tag="eq")
        nc.vector.tensor_tensor(out=eq[:], in0=iota_b,
                                in1=posb.to_broadcast([128, GT, 128]),
                                op=mybir.AluOpType.is_equal)
        sel = sb.tile([128, GT, 128], dt.bfloat16, tag="sel")
        nc.vector.tensor_tensor(out=sel[:], in0=eq[:],
                                in1=wvb.to_broadcast([128, GT, 128]),
                                op=mybir.AluOpType.mult)

        acc = ps.tile([128, H2], dt.float32, tag="acc128")
        nc.tensor.matmul(acc[:], lhsT=ones_t[:], rhs=b2_t[:], start=True, stop=False)
        for c in range(G_A):
            nc.tensor.matmul(acc[:], lhsT=sel[:, c, :], rhs=msgsA[:, c, :],
                             start=False, stop=False)
        for c in range(G_B):
            nc.tensor.matmul(acc[:], lhsT=sel[:, G_A + c, :], rhs=msgsB[:, c, :],
                             start=False, stop=(c == G_B - 1))

        ob = sb.tile([128, H2], dt.float32, tag="ob")
        nc.vector.tensor_copy(out=ob[:], in_=acc[:])
        nc.sync.dma_start(out=tens[p + "out"][b * 128:b * 128 + rows, :],
                          in_=ob[:rows, :])


def _build_program(GAd, GBd, GAs, GBs, N16=None):
    dt = mybir.dt
    nc = bacc.Bacc("TRN2", target_bir_lowering=False, debug=False,
                   num_devices=NCORES)
    tens = {}

    def inp(name, shape, dtype):
        tens[name] = nc.dram_tensor(name, shape, dtype, kind="ExternalInput")

    for p, (GA, GB) in (("d", (GAd, GBd)), ("s", (GAs, GBs))):
        GT = GA + GB
        inp(p + "xT", [NBLK * 128, F_IN], dt.bfloat16)
        inp(p + "W1", [F_IN, H1], dt.bfloat16)
        inp(p + "W2", [H1, H2], dt.bfloat16)
        inp(p + "b1", [1, H1], dt.bfloat16)
        inp(p + "b2", [1, H2], dt.bfloat16)
        inp(p + "idxA", [128, NBLK * GA * 8], dt.int16)
        inp(p + "idxB", [128, NBLK * GB * 8], dt.int16)
        inp(p + "pos", [128, NBLK * GT], dt.bfloat16)
        inp(p + "wv", [128, NBLK * GT], dt.bfloat16)
        tens[p + "out"] = nc.dram_tensor(p + "out", [PER_CORE, H2], dt.float32,
                                         kind="ExternalOutput")
    inp("iota", [128, 128], dt.bfloat16)
    inp("ident", [128, 128], dt.bfloat16)
    inp("ones", [1, 128], dt.bfloat16)

    with tile.TileContext(nc) as tc:
        with (
            tc.tile_pool(name="sbuf", bufs=2) as sb,
            tc.tile_pool(name="psum", bufs=2, space="PSUM") as ps,
            tc.tile_pool(name="dram", bufs=1, space="DRAM") as dr,
        ):
            iota_t = sb.tile([128, 128], dt.bfloat16, tag="iota")
            nc.sync.dma_start(out=iota_t[:], in_=tens["iota"][:])
            ident_t = sb.tile([128, 128], dt.bfloat16, tag="ident")
            nc.sync.dma_start(out=ident_t[:], in_=tens["ident"][:])
            ones_t = sb.tile([1, 128], dt.bfloat16, tag="ones")
            nc.sync.dma_start(out=ones_t[:], in_=tens["ones"][:])
            consts = (iota_t, ident_t, ones_t)

            nd = N16["d"]
            ns_ = N16["s"]
            std = _graph_setup(nc, tc, sb, ps, dr, "d", GAd, GBd, tens, consts,
                               N_A=nd[:, 0], N_B=nd[:, 1])
            sts = _graph_setup(nc, tc, sb, ps, dr, "s", GAs, GBs, tens, consts,
                               N_A=ns_[:, 0], N_B=ns_[:, 1])
            _phase_A(std)
            _phase_A(sts)
            _phase_B(std)
            _phase_B(sts)
            _phase_C(std)
            _phase_C(sts)
    return nc


# ----------------------------------------------------------------------------
# Entry point
# ----------------------------------------------------------------------------

def kernel(drug_x, dis_x, drug_src, drug_dst, drug_w,
           dis_src, dis_dst, dis_w,
           W1d, b1d, W2d, b2d, W1s, b1s, W2s, b2s,
           _run_opts=None):
    graphs = {
        "d": (drug_x, drug_src, drug_dst, drug_w, W1d, b1d, W2d, b2d),
        "s": (dis_x, dis_src, dis_dst, dis_w, W1s, b1s, W2s, b2s),
    }
    G = {}
    preps = {}
    for p, (x, src, dst, w, W1, b1, W2, b2) in graphs.items():
        src = np.asarray(src); dst = np.asarray(dst); w = np.asarray(w)
        mA, mB = _analyze(src, dst)
        GA, GB = -(-mA // 128), -(-mB // 128)
        G[p] = (GA, GB)
        G[p + "16"] = _analyze_blocks(src, dst)
        preps[p] = _prep_graph(src, dst, w, GA, GB)

    nc = _build_program(G["d"][0], G["d"][1], G["s"][0], G["s"][1],
                        N16={"d": G["d16"], "s": G["s16"]})
    nc.compile()

    base = {
        "iota": np.tile(np.arange(128, dtype=np.float32)[None, :].astype(BF16), (128, 1)),
        "ident": np.eye(128, dtype=np.float32).astype(BF16),
        "ones": np.ones((1, 128), BF16),
    }
    for p, (x, src, dst, w, W1, b1, W2, b2) in graphs.items():
        base[p + "W1"] = np.asarray(W1).astype(BF16)
        base[p + "W2"] = np.asarray(W2).astype(BF16)
        base[p + "b1"] = np.asarray(b1).astype(BF16)[None, :]
        base[p + "b2"] = np.asarray(b2).astype(BF16)[None, :]

    in_maps = []
    for k in range(NCORES):
        m = dict(base)
        for p, (x, *_rest) in graphs.items():
            m[p + "xT"] = _prep_x(np.asarray(x), k)
            m.update({p + n: preps[p][k][n] for n in ("idxA", "idxB", "pos", "wv")})
        in_maps.append(m)

    res = run_bass_kernel_spmd(nc, in_maps, core_ids=list(range(NCORES)),
                               **(_run_opts or {}))
    emb1 = np.concatenate([res.results[k]["dout"] for k in range(NCORES)], axis=0)
    emb2 = np.concatenate([res.results[k]["sout"] for k in range(NCORES)], axis=0)
    if _run_opts:
        kernel.last_results = res
    return emb1, emb2



# revision 3
# speedup vs baseline: 2.7470x; 2.7470x over previous
"""Trainium2 Bass kernel for the 2-graph GCN (nn_Net_39041252721058).

Strategy (8 NeuronCores, SPMD single program):
  - Core k owns dst nodes [k*6250, (k+1)*6250). All edges with dst in that
    range are processed by core k, grouped by 128-node dst blocks.
  - Layer math uses the linearity of spmm: project first (x@W1 row-sharded,
    AllGather of the projected bf16 table), then per-edge gather rows of the
    table with SWDGE dma_gather (int16 indices; src split in two classes at
    row 32768 so indices fit), then segment-sum via one-hot matmuls on the
    tensor engine accumulating in PSUM (edges on the contraction axis,
    one-hot dst-slot selection matrices built on the vector engine).
  - h = relu(agg + b1) stays on-chip per block; support2 = h @ W2 uses PE
    transposes of h; AllGather of support2, then the same gather/scatter
    machinery for layer 2.
  - The two graphs' phases are interleaved (A_d, AG1_d, A_s, AG1_s, B_d,
    AG2_d, B_s, AG2_s, C_d, C_s) so all collectives except the first hide
    behind the other graph's gather stream.
  - Phase A interleaves two PSUM accumulators (even/odd k-tiles) so PE
    weight loads pipeline; the halves are combined via a scalar-engine
    PSUM->SBUF copy plus a vector add (a vector op may read only one PSUM
    operand).

Perf model (measured on trn2): the kernel is bound by the SWDGE gather
ucode/ring, ~135 ns per 16-row ring entry plus ~0.7 us per dma_gather
call, insensitive to row bytes and to index order. Optimizations applied,
in order of measured impact:
  - gather calls batched to 1536 rows (96 of the ring's 128 entries;
    >=184 entries crashes the ucode, so one call per class is impossible),
  - per-BLOCK gather counts (ceil-16 of the max count over the 8 cores,
    computed at compile time; the SPMD constraint is across cores, not
    across the unrolled block loop),
  - graph-interleaved phases, dual-PSUM phase A, single_packet=False
    (True crashes the ucode at 1536-row calls; it only works at 512).
Rows gathered are ~3.5%% above the true edge count; driving num_idxs_reg
from per-core registers to close that gap crashes the runtime (the
completion-semaphore contract appears to assume the static count).
Everything else (PE one-hot matmuls, vector sel builds, collectives 2-4)
overlaps fully under the gather stream; the only exposed non-gather time
is phase A_d plus the first AllGather (~150 us).

Correctness details: pads carry w=0 so the one-hot multiply zeroes them;
message buffers are memset once on first use so never-gathered pad slots
cannot inject NaN (0*NaN = NaN would poison PSUM).
"""
import numpy as np
import ml_dtypes

import concourse.bass as bass
import concourse.bacc as bacc
import concourse.mybir as mybir
import concourse.tile as tile
from concourse.bass_utils import run_bass_kernel_spmd

NCORES = 8
NODES = 50000
PER_CORE = NODES // NCORES           # 6250
NBLK = (PER_CORE + 127) // 128       # 49 (last block has 106 nodes)
SPLIT = 32768                        # int16 gather-index class boundary
F_IN = 512
H1 = 256
H2 = 128

BF16 = ml_dtypes.bfloat16


def _set_dims(nodes, split):
    """Debug hook: shrink the problem (node count) keeping the structure."""
    global NODES, PER_CORE, NBLK, SPLIT
    NODES = nodes
    PER_CORE = NODES // NCORES
    NBLK = (PER_CORE + 127) // 128
    SPLIT = split


# ----------------------------------------------------------------------------
# Host-side edge preprocessing
# ----------------------------------------------------------------------------

def _analyze(src, dst):
    core = dst // PER_CORE
    blk = (dst % PER_CORE) // 128
    cls = (src >= SPLIT).astype(np.int64)
    key = (core * NBLK + blk) * 2 + cls
    counts = np.bincount(key, minlength=NCORES * NBLK * 2).reshape(-1, 2)
    return int(counts[:, 0].max()), int(counts[:, 1].max())


def _analyze_blocks(src, dst):
    """Per-block max-over-cores counts, ceil-16, per class: [NBLK, 2] int."""
    core = dst // PER_CORE
    blk = (dst % PER_CORE) // 128
    cls = (src >= SPLIT).astype(np.int64)
    key = (core * NBLK + blk) * 2 + cls
    counts = np.bincount(key, minlength=NCORES * NBLK * 2)
    counts = counts.reshape(NCORES, NBLK, 2).max(axis=0)
    return np.maximum(16, -(-counts // 16) * 16)


def _prep_graph(src, dst, w, G_A, G_B):
    """Per-core gather index / one-hot position / weight arrays (padded)."""
    GT = G_A + G_B
    core_all = dst // PER_CORE
    out = []
    for k in range(NCORES):
        m = core_all == k
        s, ww = src[m], w[m]
        rel = dst[m] - k * PER_CORE
        blk = rel // 128
        slot = rel % 128
        cls = (s >= SPLIT).astype(np.int64)
        order = np.lexsort((s, cls, blk))
        s, ww, blk, slot, cls = s[order], ww[order], blk[order], slot[order], cls[order]

        idxA = np.zeros((NBLK, G_A * 128), np.int16)
        idxB = np.zeros((NBLK, G_B * 128), np.int16)
        pos = np.zeros((NBLK, GT * 128), np.float32)
        wv = np.zeros((NBLK, GT * 128), np.float32)
        key = blk * 2 + cls
        cnt = np.bincount(key, minlength=NBLK * 2).reshape(NBLK, 2)
        assert cnt[:, 0].max() <= G_A * 128 and cnt[:, 1].max() <= G_B * 128
        starts = np.concatenate([[0], np.cumsum(cnt.ravel())])
        for b in range(NBLK):
            nA, nB = cnt[b, 0], cnt[b, 1]
            oA, oB = starts[b * 2], starts[b * 2 + 1]
            idxA[b, :nA] = s[oA:oA + nA]
            idxB[b, :nB] = s[oB:oB + nB] - SPLIT
            pos[b, :nA] = slot[oA:oA + nA]
            pos[b, G_A * 128:G_A * 128 + nB] = slot[oB:oB + nB]
            wv[b, :nA] = ww[oA:oA + nA]
            wv[b, G_A * 128:G_A * 128 + nB] = ww[oB:oB + nB]

        def wrap_idx(a):
            nb, n = a.shape
            b16 = a.reshape(nb, n // 16, 16).transpose(2, 0, 1).reshape(16, -1)
            return np.tile(b16, (8, 1)).copy()

        def wrap_edge(a):
            nb, n = a.shape
            return (a.reshape(nb, n // 128, 128).transpose(2, 0, 1)
                    .reshape(128, -1).astype(BF16).copy())

        out.append({
            "idxA": wrap_idx(idxA),
            "idxB": wrap_idx(idxB),
            "pos": wrap_edge(pos),
            "wv": wrap_edge(wv),
        })
    return out


def _prep_x(x, k):
    """Blocked transposed node features for core k: [NBLK*128, F_IN] bf16
    with row b*128+i, col kt*128+j = x[k*PER_CORE + b*128 + j, kt*128 + i]."""
    xs = np.zeros((NBLK * 128, F_IN), BF16)
    xk = x[k * PER_CORE:(k + 1) * PER_CORE].astype(BF16)  # [6250, 512]
    for b in range(NBLK):
        rows = min(128, PER_CORE - b * 128)
        blkT = xk[b * 128:b * 128 + rows].T  # [512, rows]
        t = blkT.reshape(4, 128, rows)       # [kt, i, j]
        xs[b * 128:(b + 1) * 128, :] = np.transpose(
            np.pad(t, ((0, 0), (0, 0), (0, 128 - rows))), (1, 0, 2)
        ).reshape(128, 512)
    return xs


# ----------------------------------------------------------------------------
# Device program
# ----------------------------------------------------------------------------

def _chunks16(total):
    """Split `total` (multiple of 16) gather rows into calls of <=1536 rows
    (96 ring entries of 16 rows; the SWDGE ring holds 128)."""
    out, r0 = [], 0
    while r0 < total:
        n = min(1536, total - r0)
        out.append((r0, n))
        r0 += n
    return out


class _QueueRR:
    """Round-robin SWDGE queue assignment across gather calls. Queue q's
    descriptors are generated by Q7 core pair (2q, 2q+1), so calls on
    different queues generate concurrently."""
    def __init__(self, n):
        self.n = n
        self.i = 0

    def next(self):
        q = self.i % self.n
        self.i += 1
        return q


NQUEUES = 4
_qrr = _QueueRR(NQUEUES)


def _graph_setup(nc, tc, sb, ps, dr, p, G_A, G_B, tens, consts,
                 N_A=None, N_B=None):
    """Load resident tiles + alloc DRAM intermediates for one graph.
    N_A/N_B: per-block gather row counts [NBLK] (ceil-16 max over cores)."""
    if N_A is None:
        N_A = [G_A * 128] * NBLK
        N_B = [G_B * 128] * NBLK
    GT = G_A + G_B
    dt = mybir.dt
    iota_t, ident_t, ones_t = consts

    # resident per-graph tiles
    w1_t = sb.tile([128, 4, H1], dt.bfloat16, tag="w1")
    nc.sync.dma_start(out=w1_t[:], in_=tens[p + "W1"][:].rearrange("(a b) c -> b a c", b=128))
    w2_t = sb.tile([128, 2, H2], dt.bfloat16, tag="w2")
    nc.sync.dma_start(out=w2_t[:], in_=tens[p + "W2"][:].rearrange("(a b) c -> b a c", b=128))
    b1_t = sb.tile([1, H1], dt.bfloat16, tag="b1")
    nc.sync.dma_start(out=b1_t[:], in_=tens[p + "b1"][:])
    b2_t = sb.tile([1, H2], dt.bfloat16, tag="b2")
    nc.sync.dma_start(out=b2_t[:], in_=tens[p + "b2"][:])
    idxA_t = sb.tile([128, NBLK * G_A * 8], dt.int16, tag="idxA")
    nc.sync.dma_start(out=idxA_t[:], in_=tens[p + "idxA"][:])
    idxB_t = sb.tile([128, NBLK * G_B * 8], dt.int16, tag="idxB")
    nc.sync.dma_start(out=idxB_t[:], in_=tens[p + "idxB"][:])
    wv_t = sb.tile([128, NBLK * GT], dt.bfloat16, tag="wv")
    nc.sync.dma_start(out=wv_t[:], in_=tens[p + "wv"][:])
    pos_t = sb.tile([128, NBLK * GT], dt.bfloat16, tag="pos")
    nc.sync.dma_start(out=pos_t[:], in_=tens[p + "pos"][:])

    # DRAM intermediates
    s1_own = dr.tile([PER_CORE, H1], dt.bfloat16, tag=p + "s1o")
    s1_full = dr.tile([NODES, H1], dt.bfloat16, tag=p + "s1f", addr_space="Shared")
    s2_own = dr.tile([PER_CORE, H2], dt.bfloat16, tag=p + "s2o")
    s2_full = dr.tile([NODES, H2], dt.bfloat16, tag=p + "s2f", addr_space="Shared")

    return dict(locals())


def _phase_A(st):
    nc, sb, ps, p, tens = st["nc"], st["sb"], st["ps"], st["p"], st["tens"]
    dt = mybir.dt
    w1_t, s1_own = st["w1_t"], st["s1_own"]
    # ---- Phase A: support1 = x @ W1 (own rows) ----
    for b in range(NBLK):
        rows = min(128, PER_CORE - b * 128)
        xt = sb.tile([128, 4, 128], dt.bfloat16, tag="xt")
        nc.sync.dma_start(
            out=xt[:],
            in_=tens[p + "xT"][b * 128:(b + 1) * 128, :].rearrange("p (a c) -> p a c", a=4),
        )
        accE = ps.tile([128, H1], dt.float32, tag="acc256")
        accO = ps.tile([128, H1], dt.float32, tag="acc256b")
        for kt in range(4):
            a = accE if kt % 2 == 0 else accO
            nc.tensor.matmul(a[:], lhsT=xt[:, kt, :], rhs=w1_t[:, kt, :],
                             start=(kt < 2), stop=(kt >= 2))
        tA = sb.tile([128, H1], dt.bfloat16, tag="tA", name="tA")
        nc.scalar.activation(tA[:], accO[:], mybir.ActivationFunctionType.Copy)
        s1sb = sb.tile([128, H1], dt.bfloat16, tag="s1sb")
        nc.vector.tensor_tensor(out=s1sb[:], in0=accE[:], in1=tA[:],
                                op=mybir.AluOpType.add)
        nc.sync.dma_start(out=s1_own[b * 128:b * 128 + rows, :], in_=s1sb[:rows, :])

    nc.gpsimd.collective_compute(
        "AllGather", mybir.AluOpType.bypass,
        replica_groups=[list(range(NCORES))],
        ins=[s1_own.opt()], outs=[st["s1_full"].opt()],
    )


def _phase_B(st):
    nc, sb, ps, p, tens = st["nc"], st["sb"], st["ps"], st["p"], st["tens"]
    dt = mybir.dt
    G_A, G_B, GT = st["G_A"], st["G_B"], st["GT"]
    N_A, N_B = st["N_A"], st["N_B"]
    iota_t, ident_t, ones_t = st["iota_t"], st["ident_t"], st["ones_t"]
    idxA_t, idxB_t, pos_t, wv_t = st["idxA_t"], st["idxB_t"], st["pos_t"], st["wv_t"]
    w2_t, b1_t = st["w2_t"], st["b1_t"]
    s1_full, s2_own = st["s1_full"], st["s2_own"]
    # ---- Phase B: agg1 -> h -> support2 (own rows) ----
    for b in range(NBLK):
        rows = min(128, PER_CORE - b * 128)
        msgsA = sb.tile([128, G_A, H1], dt.bfloat16, tag="mA")
        if b < 2:
            nc.vector.memset(msgsA[:], 0.0)
        for r0, n in _chunks16(int(N_A[b])):
            g0, g1 = r0 // 128, (r0 + n + 127) // 128
            nc.gpsimd.dma_gather(
                msgsA[:, g0:g1, :], s1_full[:],
                idxA_t[:, b * G_A * 8 + r0 // 16:
                       b * G_A * 8 + r0 // 16 + -(-n // 16)],
                n, n, H1, single_packet=False, queue_num=_qrr.next())
        msgsB = sb.tile([128, G_B, H1], dt.bfloat16, tag="mB")
        if b < 2:
            nc.vector.memset(msgsB[:], 0.0)
        for r0, n in _chunks16(int(N_B[b])):
            g0, g1 = r0 // 128, (r0 + n + 127) // 128
            nc.gpsimd.dma_gather(
                msgsB[:, g0:g1, :], s1_full[SPLIT:, :],
                idxB_t[:, b * G_B * 8 + r0 // 16:
                       b * G_B * 8 + r0 // 16 + -(-n // 16)],
                n, n, H1, single_packet=False, queue_num=_qrr.next())

        posb = pos_t[:, b * GT:(b + 1) * GT]
        wvb = wv_t[:, b * GT:(b + 1) * GT]
        ia = iota_t[:, :]
        iota_b = bass.AP(tensor=ia.tensor, offset=ia.offset,
                         ap=[ia.ap[0], [0, GT], ia.ap[1]])
        eq = sb.tile([128, GT, 128], dt.bfloat16, tag="eq")
        nc.vector.tensor_tensor(out=eq[:], in0=iota_b,
                                in1=posb.to_broadcast([128, GT, 128]),
                                op=mybir.AluOpType.is_equal)
        sel = sb.tile([128, GT, 128], dt.bfloat16, tag="sel")
        nc.vector.tensor_tensor(out=sel[:], in0=eq[:],
                                in1=wvb.to_broadcast([128, GT, 128]),
                                op=mybir.AluOpType.mult)

        acc = ps.tile([128, H1], dt.float32, tag="acc256")
        nc.tensor.matmul(acc[:], lhsT=ones_t[:], rhs=b1_t[:], start=True, stop=False)
        for c in range(G_A):
            nc.tensor.matmul(acc[:], lhsT=sel[:, c, :], rhs=msgsA[:, c, :],
                             start=False, stop=False)
        for c in range(G_B):
            nc.tensor.matmul(acc[:], lhsT=sel[:, G_A + c, :], rhs=msgsB[:, c, :],
                             start=False, stop=(c == G_B - 1))

        h_bf = sb.tile([128, H1], dt.bfloat16, tag="hbf")
        nc.scalar.activation(h_bf[:], acc[:], mybir.ActivationFunctionType.Relu)

        sp2 = ps.tile([128, H2], dt.float32, tag="acc128")
        for half in range(2):
            tp = ps.tile([128, 128], dt.bfloat16, tag="tp")
            nc.tensor.transpose(out=tp[:], in_=h_bf[:, half * 128:(half + 1) * 128],
                                identity=ident_t[:])
            hT = sb.tile([128, 128], dt.bfloat16, tag="hT")
            nc.vector.tensor_copy(out=hT[:], in_=tp[:])
            nc.tensor.matmul(sp2[:], lhsT=hT[:], rhs=w2_t[:, half, :],
                             start=(half == 0), stop=(half == 1))
        s2sb = sb.tile([128, H2], dt.bfloat16, tag="s2sb")
        nc.vector.tensor_copy(out=s2sb[:], in_=sp2[:])
        nc.sync.dma_start(out=s2_own[b * 128:b * 128 + rows, :], in_=s2sb[:rows, :])

    nc.gpsimd.collective_compute(
        "AllGather", mybir.AluOpType.bypass,
        replica_groups=[list(range(NCORES))],
        ins=[s2_own.opt()], outs=[st["s2_full"].opt()],
    )


def _phase_C(st):
    nc, sb, ps, p, tens = st["nc"], st["sb"], st["ps"], st["p"], st["tens"]
    dt = mybir.dt
    G_A, G_B, GT = st["G_A"], st["G_B"], st["GT"]
    N_A, N_B = st["N_A"], st["N_B"]
    iota_t, ones_t = st["iota_t"], st["ones_t"]
    idxA_t, idxB_t, pos_t, wv_t = st["idxA_t"], st["idxB_t"], st["pos_t"], st["wv_t"]
    b2_t = st["b2_t"]
    s2_full = st["s2_full"]
    # ---- Phase C: agg2 + b2 -> out ----
    for b in range(NBLK):
        rows = min(128, PER_CORE - b * 128)
        msgsA = sb.tile([128, G_A, H2], dt.bfloat16, tag="mA")
        if b < 2:
            nc.vector.memset(msgsA[:], 0.0)
        for r0, n in _chunks16(int(N_A[b])):
            g0, g1 = r0 // 128, (r0 + n + 127) // 128
            nc.gpsimd.dma_gather(
                msgsA[:, g0:g1, :], s2_full[:],
                idxA_t[:, b * G_A * 8 + r0 // 16:
                       b * G_A * 8 + r0 // 16 + -(-n // 16)],
                n, n, H2, single_packet=False, queue_num=_qrr.next())
        msgsB = sb.tile([128, G_B, H2], dt.bfloat16, tag="mB")
        if b < 2:
            nc.vector.memset(msgsB[:], 0.0)
        for r0, n in _chunks16(int(N_B[b])):
            g0, g1 = r0 // 128, (r0 + n + 127) // 128
            nc.gpsimd.dma_gather(
                msgsB[:, g0:g1, :], s2_full[SPLIT:, :],
                idxB_t[:, b * G_B * 8 + r0 // 16:
                       b * G_B * 8 + r0 // 16 + -(-n // 16)],
                n, n, H2, single_packet=False, queue_num=_qrr.next())

        posb = pos_t[:, b * GT:(b + 1) * GT]
        wvb = wv_t[:, b * GT:(b + 1) * GT]
        ia = iota_t[:, :]
        iota_b = bass.AP(tensor=ia.tensor, offset=ia.offset,
                         ap=[ia.ap[0], [0, GT], ia.ap[1]])
        eq = sb.tile([128, GT, 128], dt.bfloat16, tag="eq")
        nc.vector.tensor_tensor(out=eq[:], in0=iota_b,
                                in1=posb.to_broadcast([128, GT, 128]),
                                op=mybir.AluOpType.is_equal)
        sel = sb.tile([128, GT, 128], dt.bfloat16, tag="sel")
        nc.vector.tensor_tensor(out=sel[:], in0=eq[:],
                                in1=wvb.to_broadcast([128, GT, 128]),
                                op=mybir.AluOpType.mult)

        acc = ps.tile([128, H2], dt.float32, tag="acc128")
        nc.tensor.matmul(acc[:], lhsT=ones_t[:], rhs=b2_t[:], start=True, stop=False)
        for c in range(G_A):
            nc.tensor.matmul(acc[:], lhsT=sel[:, c, :], rhs=msgsA[:, c, :],
                             start=False, stop=False)
        for c in range(G_B):
            nc.tensor.matmul(acc[:], lhsT=sel[:, G_A + c, :], rhs=msgsB[:, c, :],
                             start=False, stop=(c == G_B - 1))

        ob = sb.tile([128, H2], dt.float32, tag="ob")
        nc.vector.tensor_copy(out=ob[:], in_=acc[:])
        nc.sync.dma_start(out=tens[p + "out"][b * 128:b * 128 + rows, :],
                          in_=ob[:rows, :])


def _build_program(GAd, GBd, GAs, GBs, N16=None):
    dt = mybir.dt
    nc = bacc.Bacc("TRN2", target_bir_lowering=False, debug=False,
                   num_devices=NCORES, num_swdge_queues=NQUEUES)
    tens = {}

    def inp(name, shape, dtype):
        tens[name] = nc.dram_tensor(name, shape, dtype, kind="ExternalInput")

    for p, (GA, GB) in (("d", (GAd, GBd)), ("s", (GAs, GBs))):
        GT = GA + GB
        inp(p + "xT", [NBLK * 128, F_IN], dt.bfloat16)
        inp(p + "W1", [F_IN, H1], dt.bfloat16)
        inp(p + "W2", [H1, H2], dt.bfloat16)
        inp(p + "b1", [1, H1], dt.bfloat16)
        inp(p + "b2", [1, H2], dt.bfloat16)
        inp(p + "idxA", [128, NBLK * GA * 8], dt.int16)
        inp(p + "idxB", [128, NBLK * GB * 8], dt.int16)
        inp(p + "pos", [128, NBLK * GT], dt.bfloat16)
        inp(p + "wv", [128, NBLK * GT], dt.bfloat16)
        tens[p + "out"] = nc.dram_tensor(p + "out", [PER_CORE, H2], dt.float32,
                                         kind="ExternalOutput")
    inp("iota", [128, 128], dt.bfloat16)
    inp("ident", [128, 128], dt.bfloat16)
    inp("ones", [1, 128], dt.bfloat16)

    with tile.TileContext(nc) as tc:
        with (
            tc.tile_pool(name="sbuf", bufs=2) as sb,
            tc.tile_pool(name="psum", bufs=2, space="PSUM") as ps,
            tc.tile_pool(name="dram", bufs=1, space="DRAM") as dr,
        ):
            iota_t = sb.tile([128, 128], dt.bfloat16, tag="iota")
            nc.sync.dma_start(out=iota_t[:], in_=tens["iota"][:])
            ident_t = sb.tile([128, 128], dt.bfloat16, tag="ident")
            nc.sync.dma_start(out=ident_t[:], in_=tens["ident"][:])
            ones_t = sb.tile([1, 128], dt.bfloat16, tag="ones")
            nc.sync.dma_start(out=ones_t[:], in_=tens["ones"][:])
            consts = (iota_t, ident_t, ones_t)

            nd = N16["d"]
            ns_ = N16["s"]
            std = _graph_setup(nc, tc, sb, ps, dr, "d", GAd, GBd, tens, consts,
                               N_A=nd[:, 0], N_B=nd[:, 1])
            sts = _graph_setup(nc, tc, sb, ps, dr, "s", GAs, GBs, tens, consts,
                               N_A=ns_[:, 0], N_B=ns_[:, 1])
            _phase_A(std)
            _phase_A(sts)
            _phase_B(std)
            _phase_B(sts)
            _phase_C(std)
            _phase_C(sts)
    return nc


# ----------------------------------------------------------------------------
# Entry point
# ----------------------------------------------------------------------------

def kernel(drug_x, dis_x, drug_src, drug_dst, drug_w,
           dis_src, dis_dst, dis_w,
           W1d, b1d, W2d, b2d, W1s, b1s, W2s, b2s,
           _run_opts=None):
    graphs = {
        "d": (drug_x, drug_src, drug_dst, drug_w, W1d, b1d, W2d, b2d),
        "s": (dis_x, dis_src, dis_dst, dis_w, W1s, b1s, W2s, b2s),
    }
    G = {}
    preps = {}
    for p, (x, src, dst, w, W1, b1, W2, b2) in graphs.items():
        src = np.asarray(src); dst = np.asarray(dst); w = np.asarray(w)
        mA, mB = _analyze(src, dst)
        GA, GB = -(-mA // 128), -(-mB // 128)
        G[p] = (GA, GB)
        G[p + "16"] = _analyze_blocks(src, dst)
        preps[p] = _prep_graph(src, dst, w, GA, GB)

    nc = _build_program(G["d"][0], G["d"][1], G["s"][0], G["s"][1],
                        N16={"d": G["d16"], "s": G["s16"]})
    nc.compile()

    base = {
        "iota": np.tile(np.arange(128, dtype=np.float32)[None, :].astype(BF16), (128, 1)),
        "ident": np.eye(128, dtype=np.float32).astype(BF16),
        "ones": np.ones((1, 128), BF16),
    }
    for p, (x, src, dst, w, W1, b1, W2, b2) in graphs.items():
        base[p + "W1"] = np.asarray(W1).astype(BF16)
        base[p + "W2"] = np.asarray(W2).astype(BF16)
        base[p + "b1"] = np.asarray(b1).astype(BF16)[None, :]
        base[p + "b2"] = np.asarray(b2).astype(BF16)[None, :]

    in_maps = []
    for k in range(NCORES):
        m = dict(base)
        for p, (x, *_rest) in graphs.items():
            m[p + "xT"] = _prep_x(np.asarray(x), k)
            m.update({p + n: preps[p][k][n] for n in ("idxA", "idxB", "pos", "wv")})
        in_maps.append(m)

    res = run_bass_kernel_spmd(nc, in_maps, core_ids=list(range(NCORES)),
                               **(_run_opts or {}))
    emb1 = np.concatenate([res.results[k]["dout"] for k in range(NCORES)], axis=0)
    emb2 = np.concatenate([res.results[k]["sout"] for k in range(NCORES)], axis=0)
    if _run_opts:
        kernel.last_results = res
    return emb1, emb2

